# revision 17
# baseline (speedup 1.0000x reference)
"""Trainium2 Bass kernel for a dense multi-head attention layer.

Problem (hardcoded shapes):
    hidden_states [2, 2048, 2048] fp32, attention_mask [2,1,1,2048] int32 (all ones),
    Wq/Wk/Wv/Wo [2048, 2048] fp32, biases [2048] fp32 (zeros in practice).
    out = MHA(hidden) with H=16 heads, head_dim=128.

Sharding: 8 cores = 2 batches x 4 head-groups (4 heads per core, tensor
parallel over heads). Each core computes q/k/v projections for its 4 heads,
attention, and a partial output projection; the host sums the 4 partials per
batch.

All on-chip data is bf16 (validated: end-to-end rel err ~7e-3 vs the fp32
reference, threshold 2e-2). bf16 matmuls run at the same 1 cycle/row as
fp32r on the PE but halve every SBUF footprint and DMA byte count, which
lets the whole working set (hidden slab, weights, q/k/v, exp tiles) stay
on-chip -- no DRAM round-trip for intermediates at all.

Layout: everything is arranged so no on-device transpose is ever needed.
The host supplies hidden^T and pre-transposed weights; scores are computed
keys-major (sT = kT^T @ qT) so the PV matmul consumes exp(sT) directly and
produces attn^T, which is exactly the lhsT layout the output projection
wants. Softmax denominators come from a ones-matrix matmul accumulated in
PSUM (broadcast across partitions for free); normalization happens in the
DVE multiply that evicts the PV accumulator.
"""
import os
import sys

if "/opt/trn_rl_repo" not in sys.path:
    sys.path.insert(0, "/opt/trn_rl_repo")

# If a previous run crashed the NEFF execution, a fresh NRT open with this
# flag recovers the cores instead of failing with EXEC_UNIT_UNRECOVERABLE.
os.environ.setdefault("NEURON_RT_RESET_CORES", "1")

import numpy as np

B, S, D, H, HD = 2, 2048, 2048, 16, 128
NCORES = 8
GROUPS = 4            # head-groups == cores per batch
GH = H // GROUPS      # heads per core = 4
GD = GH * HD          # 512 projection cols per core
ST = 512              # s/q/o tile width (PSUM bank = 512 fp32)
NSB = S // 128        # 16 s-blocks
NEB = D // 128        # 16 e-blocks (contraction)
NST = S // ST         # 4 s-tiles
SCALE = 1.0 / float(np.sqrt(HD))

_RUNNER = {}


def _build_nc(reps=1):
    import concourse.tile as tile
    from concourse import bacc, mybir

    nc = bacc.Bacc("TRN2", target_bir_lowering=False, debug=False,
                   num_devices=NCORES)

    bf16 = mybir.dt.bfloat16
    hT = nc.dram_tensor("hT", [D, S], bf16, kind="ExternalInput")
    wqT = nc.dram_tensor("wqT", [D, GD], bf16, kind="ExternalInput")
    wkT = nc.dram_tensor("wkT", [D, GD], bf16, kind="ExternalInput")
    wvT = nc.dram_tensor("wvT", [D, GD], bf16, kind="ExternalInput")
    woT = nc.dram_tensor("woT", [GD, D], bf16, kind="ExternalInput")
    out = nc.dram_tensor("out", [S, D], bf16, kind="ExternalOutput")

    with tile.TileContext(nc) as tc:
        for rep in range(reps):
            # reps>1 is a timing aid only: bodies serialize through SBUF
            # reuse, so t(reps=3)-t(reps=1) isolates two body executions.
            _emit_body(nc, tc, tile, mybir, hT, wqT, wkT, wvT, woT, out)

    nc.compile()
    return nc


def _emit_body(nc, tc, tile, mybir, hT, wqT, wkT, wvT, woT, out):
    f32 = mybir.dt.float32
    bf16 = mybir.dt.bfloat16
    Exp = mybir.ActivationFunctionType.Exp

    if True:
        with tc.tile_pool(name="persist", bufs=1) as persist:
            # per-head attention operands, filled by phase 1, all bf16:
            #   qt_all [e, h, tok]      (rhs of scores)
            #   kt_all [e, h, kb, tok]  (lhsT of scores)
            #   vt_all [tok, kb, h, hd] (lhsT of PV)
            qt_all = persist.tile([128, GH, S], bf16)
            kt_all = persist.tile([128, GH, NSB, 128], bf16)
            vt_all = persist.tile([128, NSB, GH, HD], bf16)
            wo_sb = persist.tile([128, GH, D], bf16)
            # f32r (E8M11) runs at 1 cycle/row at free>=256, vs 4 for true
            # f32; DVE writes into f32r tiles round on store, satisfying
            # the BIR verifier's rounded-producer rule.
            f32r = mybir.dt.float32r
            ones_f = persist.tile([128, 128], f32)
            ones_sb = persist.tile([128, 128], f32r)
            nc.vector.memset(ones_f, 1.0)
            nc.vector.tensor_copy(ones_sb, ones_f)

            # ---------------- phase 1: q/k/v projections ----------------
            with tc.tile_pool(name="wqk", bufs=1) as wqk, \
                 tc.tile_pool(name="hall", bufs=1) as hall, \
                 tc.tile_pool(name="ps1", bufs=8, space="PSUM") as ps1:
                wq_sb = wqk.tile([128, NEB, GD], bf16)
                wk_sb = wqk.tile([128, NEB, GD], bf16)
                wv_sb = wqk.tile([128, NEB, GD], bf16)
                h_all = hall.tile([128, NEB, S], bf16)
                wqT_r = wqT.rearrange("(n p) d -> p n d", p=128)
                wkT_r = wkT.rearrange("(n p) d -> p n d", p=128)
                wvT_r = wvT.rearrange("(n p) d -> p n d", p=128)
                woT_r = woT.rearrange("(n p) o -> p n o", p=128)
                hT_r = hT.rearrange("(n p) s -> p n s", p=128)

                # interleave so eb=0 pieces of the q/k path arrive first
                for eb in range(NEB):
                    nc.sync.dma_start(out=h_all[:, eb, :], in_=hT_r[:, eb, :])
                    nc.sync.dma_start(out=wq_sb[:, eb, :], in_=wqT_r[:, eb, :])
                    nc.sync.dma_start(out=wk_sb[:, eb, :], in_=wkT_r[:, eb, :])
                for eb in range(NEB):
                    nc.sync.dma_start(out=wv_sb[:, eb, :], in_=wvT_r[:, eb, :])
                for cb in range(GH):
                    nc.sync.dma_start(out=wo_sb[:, cb, :], in_=woT_r[:, cb, :])

                for st in range(NST):
                    tok = slice(st * ST, (st + 1) * ST)
                    if st == 0:
                        # eb-outer over 8 live accumulators: consume input
                        # chunks in arrival order so the PE tracks the DMA
                        # stream instead of stalling per accumulation.
                        pss = {}
                        for h in range(GH):
                            for t in range(2):
                                pss[(h, t)] = ps1.tile([128, ST], f32,
                                                       tag="ps1",
                                                       name=f"psqk{h}{t}")
                        for eb in range(NEB):
                            for h in range(GH):
                                for t, w_sb in ((0, wq_sb), (1, wk_sb)):
                                    nc.tensor.matmul(
                                        pss[(h, t)],
                                        w_sb[:, eb, h * HD:(h + 1) * HD],
                                        h_all[:, eb, tok],
                                        start=(eb == 0), stop=(eb == NEB - 1))
                        for h in range(GH):
                            nc.scalar.copy(qt_all[:, h, tok], pss[(h, 0)])
                            nc.scalar.copy(kt_all[:, h, st * 4:(st + 1) * 4, :],
                                           pss[(h, 1)])
                    else:
                        for h in range(GH):
                            for w_sb in (wq_sb, wk_sb):
                                ps = ps1.tile([128, ST], f32, tag="ps1")
                                for eb in range(NEB):
                                    nc.tensor.matmul(
                                        ps, w_sb[:, eb, h * HD:(h + 1) * HD],
                                        h_all[:, eb, tok],
                                        start=(eb == 0), stop=(eb == NEB - 1))
                                if w_sb is wq_sb:
                                    nc.scalar.copy(qt_all[:, h, tok], ps)
                                else:
                                    nc.scalar.copy(
                                        kt_all[:, h, st * 4:(st + 1) * 4, :],
                                        ps)
                    for j in range(ST // 128):
                        ps = ps1.tile([128, GD], f32, tag="ps1")
                        for eb in range(NEB):
                            nc.tensor.matmul(
                                ps,
                                h_all[:, eb, st * ST + j * 128:
                                      st * ST + (j + 1) * 128],
                                wv_sb[:, eb, :],
                                start=(eb == 0), stop=(eb == NEB - 1))
                        nc.scalar.copy(vt_all[:, st * 4 + j, :, :], ps)

            # ------- phase 2+3: attention fused with output projection -------
            # qt outer / heads inner: the output projection for query tile qt
            # runs as soon as all heads finished that tile, filling the PE
            # while the (ACT-bound) exp stream of the next tile runs.
            with tc.tile_pool(name="expp", bufs=2) as expp, \
                 tc.tile_pool(name="sm", bufs=2) as sm, \
                 tc.tile_pool(name="attn2", bufs=2) as attn2, \
                 tc.tile_pool(name="ev3", bufs=2) as ev3, \
                 tc.tile_pool(name="ps_s", bufs=2, space="PSUM") as ps_s, \
                 tc.tile_pool(name="acc", bufs=4, space="PSUM") as acc:

                def ph3_block(prev, qt_prev, j):
                    # output projection for query tile qt_prev, s-block j
                    sb = qt_prev * (ST // 128) + j
                    ov = ev3.tile([128, D], bf16, tag="ov", name=f"ov{sb}")
                    for ot in range(NST):
                        po = acc.tile([128, ST], f32, tag="acc",
                                      name=f"po{sb}{ot}")
                        for cb in range(GH):
                            nc.tensor.matmul(
                                po,
                                prev[cb][:, j * 128:(j + 1) * 128],
                                wo_sb[:, cb, ot * ST:(ot + 1) * ST],
                                start=(cb == 0), stop=(cb == GH - 1))
                        nc.scalar.copy(ov[:, ot * ST:(ot + 1) * ST], po)
                    nc.sync.dma_start(
                        out=out[sb * 128:(sb + 1) * 128, :], in_=ov)

                pending = []

                def flush_den():
                    # denominator finish for the previous head, delayed so
                    # the DVE es-chain drains behind this head's first
                    # score pairs instead of stalling the PE.
                    es, pa, at = pending.pop()
                    den = acc.tile([128, ST], f32, tag="acc", name="den")
                    nc.tensor.matmul(den, ones_sb, es, start=True, stop=True)
                    brc = sm.tile([128, ST], f32, tag="brc")
                    nc.vector.reciprocal(brc, den)
                    nc.vector.tensor_mul(at, pa, brc)

                prev_attn = None
                for qt in range(NST):
                    attn_t = []
                    for h in range(GH):
                        qs = qt_all[:, h, qt * ST:(qt + 1) * ST]
                        expt = expp.tile([128, NSB, ST], bf16, tag="expt")
                        pa = acc.tile([128, ST], f32, tag="acc", name="pa")
                        es = sm.tile([128, ST], f32r, tag="es")

                        # software-pipelined: scores for pair kb2+1 are
                        # enqueued before the exp-consumers of pair kb2 so
                        # the PE never waits on the ACT round-trip.
                        def scores_pair(kb2):
                            ps = ps_s.tile([128, 2, ST], f32, tag="ps")
                            for half in range(2):
                                nc.tensor.matmul(
                                    ps[:, half, :],
                                    kt_all[:, h, kb2 * 2 + half, :], qs,
                                    start=True, stop=True)
                            return ps

                        ps_prev = scores_pair(0)
                        for kb2 in range(NSB // 2):
                            kbo = kb2 * 2
                            ps_next = (scores_pair(kb2 + 1)
                                       if kb2 + 1 < NSB // 2 else None)
                            pair = expt[:, kbo:kbo + 2, :]
                            nc.scalar.activation(pair, ps_prev, Exp,
                                                 scale=SCALE)
                            if kb2 == 0 and pending:
                                flush_den()
                            for half in range(2):
                                kb = kbo + half
                                if kb == 0:
                                    nc.vector.tensor_copy(es, expt[:, kb, :])
                                else:
                                    nc.vector.tensor_add(es,
                                                         es.bitcast(f32),
                                                         expt[:, kb, :])
                                nc.tensor.matmul(pa, vt_all[:, kb, h, :],
                                                 expt[:, kb, :],
                                                 start=(kb == 0),
                                                 stop=(kb == NSB - 1))
                            ps_prev = ps_next
                        at = attn2.tile([128, ST], bf16, tag=f"at{h}",
                                        name=f"at{h}{qt}")
                        pending.append((es, pa, at))
                        attn_t.append(at)
                        if prev_attn is not None:
                            ph3_block(prev_attn, qt - 1, h)
                    prev_attn = attn_t
                flush_den()
                for j in range(ST // 128):
                    ph3_block(prev_attn, NST - 1, j)


def _get_runner(reps=1):
    if reps not in _RUNNER:
        _RUNNER[reps] = _build_nc(reps)
    return _RUNNER[reps]


def _bf16(x: np.ndarray) -> np.ndarray:
    import ml_dtypes
    return np.ascontiguousarray(x).astype(ml_dtypes.bfloat16)


def _prepare_in_maps(hidden_states, Wq, Wk, Wv, Wo):
    hidden = np.asarray(hidden_states, dtype=np.float32)
    hT = [_bf16(hidden[b].T) for b in range(B)]
    wq = np.asarray(Wq, dtype=np.float32)
    wk = np.asarray(Wk, dtype=np.float32)
    wv = np.asarray(Wv, dtype=np.float32)
    wo = np.asarray(Wo, dtype=np.float32)
    in_maps = []
    for core in range(NCORES):
        b, g = divmod(core, GROUPS)
        rows = slice(g * GD, (g + 1) * GD)
        in_maps.append({
            "hT": hT[b],
            "wqT": _bf16(wq[rows, :].T),
            "wkT": _bf16(wk[rows, :].T),
            "wvT": _bf16(wv[rows, :].T),
            "woT": _bf16(wo[:, rows].T),
        })
    return in_maps


def _run_device(in_maps, trace=False):
    from concourse.bass_utils import run_bass_kernel_spmd
    nc = _get_runner()
    try:
        return run_bass_kernel_spmd(nc, in_maps, core_ids=list(range(NCORES)),
                                    trace=trace)
    except Exception:
        # Transient device failures (rare) are recoverable by reopening the
        # backend with NEURON_RT_RESET_CORES=1. Retry once.
        try:
            import jax
            jax.clear_caches()
            try:
                jax.extend.backend.clear_backends()
            except Exception:
                jax._src.api.clear_backends()
        except Exception:
            pass
        return run_bass_kernel_spmd(nc, in_maps, core_ids=list(range(NCORES)),
                                    trace=trace)


def _numpy_reference(hidden_states, attention_mask, Wq, bq, Wk, bk, Wv, bv,
                     Wo, bo):
    """Exact fallback for inputs the fast path does not handle."""
    h = np.asarray(hidden_states, dtype=np.float32)
    mask = np.asarray(attention_mask)
    q = h @ np.asarray(Wq, np.float32).T + np.asarray(bq, np.float32)
    k = h @ np.asarray(Wk, np.float32).T + np.asarray(bk, np.float32)
    v = h @ np.asarray(Wv, np.float32).T + np.asarray(bv, np.float32)
    q = q.reshape(B, S, H, HD).transpose(0, 2, 1, 3)
    k = k.reshape(B, S, H, HD).transpose(0, 2, 1, 3)
    v = v.reshape(B, S, H, HD).transpose(0, 2, 1, 3)
    scores = (q @ k.transpose(0, 1, 3, 2)).astype(np.float32) * SCALE
    scores = np.where(mask == 0, np.float32(-1e9), scores)
    scores -= scores.max(axis=-1, keepdims=True)
    probs = np.exp(scores, dtype=np.float32)
    probs /= probs.sum(axis=-1, keepdims=True)
    attn = probs @ v
    attn = attn.transpose(0, 2, 1, 3).reshape(B, S, D)
    out = attn @ np.asarray(Wo, np.float32).T + np.asarray(bo, np.float32)
    return out.astype(np.float32)


def kernel(hidden_states, attention_mask, Wq, bq, Wk, bk, Wv, bv, Wo, bo):
    mask = np.asarray(attention_mask)
    bq_np = np.asarray(bq, dtype=np.float32)
    if (mask == 0).any() or np.any(bq_np):
        # general (never hit with the reference setup_inputs): bq shifts
        # scores per-key and a masked key changes the softmax support --
        # neither is representable in the fast path's fused layout.
        return _numpy_reference(hidden_states, attention_mask, Wq, bq, Wk,
                                bk, Wv, bv, Wo, bo)

    in_maps = _prepare_in_maps(hidden_states, Wq, Wk, Wv, Wo)
    res = _run_device(in_maps)

    # bk only adds a per-query constant to scores (softmax-invariant).
    # bv passes through the probs (rows sum to 1): out += bv @ Wo.T. bo adds.
    extra = (np.asarray(bv, np.float64) @ np.asarray(Wo, np.float64).T
             + np.asarray(bo, np.float64))
    out = np.empty((B, S, D), dtype=np.float32)
    for b in range(B):
        acc = np.zeros((S, D), dtype=np.float64)
        for g in range(GROUPS):
            acc += np.asarray(res.results[b * GROUPS + g]["out"],
                              dtype=np.float64)
        out[b] = (acc + extra).astype(np.float32)
    return out


# revision 22
# speedup vs baseline: 3.7564x; 3.7564x over previous
"""Trainium2 Bass kernel for a dense multi-head attention layer.

Problem (hardcoded shapes):
    hidden_states [2, 2048, 2048] fp32, attention_mask [2,1,1,2048] int32 (all ones),
    Wq/Wk/Wv/Wo [2048, 2048] fp32, biases [2048] fp32 (zeros in practice).
    out = MHA(hidden) with H=16 heads, head_dim=128.

Sharding: 8 cores = 2 batches x 4 head-groups (4 heads per core, tensor
parallel over heads). Each core computes q/k/v projections for its 4 heads,
attention, and a partial output projection; the host sums the 4 partials per
batch.

All on-chip data is bf16 (validated: end-to-end rel err ~7e-3 vs the fp32
reference, threshold 2e-2). bf16 matmuls run at the same 1 cycle/row as
fp32r on the PE but halve every SBUF footprint and DMA byte count, which
lets the whole working set (hidden slab, weights, q/k/v, exp tiles) stay
on-chip -- no DRAM round-trip for intermediates at all.

Layout: everything is arranged so no on-device transpose is ever needed.
The host supplies hidden^T and pre-transposed weights; scores are computed
keys-major (sT = kT^T @ qT) so the PV matmul consumes exp(sT) directly and
produces attn^T, which is exactly the lhsT layout the output projection
wants. Softmax denominators come from a ones-matrix matmul accumulated in
PSUM (broadcast across partitions for free); normalization happens in the
DVE multiply that evicts the PV accumulator.
"""
import os
import sys

if "/opt/trn_rl_repo" not in sys.path:
    sys.path.insert(0, "/opt/trn_rl_repo")

# If a previous run crashed the NEFF execution, a fresh NRT open with this
# flag recovers the cores instead of failing with EXEC_UNIT_UNRECOVERABLE.
os.environ.setdefault("NEURON_RT_RESET_CORES", "1")

import numpy as np

B, S, D, H, HD = 2, 2048, 2048, 16, 128
NCORES = 8
GROUPS = 4            # head-groups == cores per batch
GH = H // GROUPS      # heads per core = 4
GD = GH * HD          # 512 projection cols per core
ST = 512              # s/q/o tile width (PSUM bank = 512 fp32)
NSB = S // 128        # 16 s-blocks
NEB = D // 128        # 16 e-blocks (contraction)
NST = S // ST         # 4 s-tiles
SCALE = 1.0 / float(np.sqrt(HD))

_RUNNER = {}


def _build_nc(reps=1):
    import concourse.tile as tile
    from concourse import bacc, mybir

    nc = bacc.Bacc("TRN2", target_bir_lowering=False, debug=False,
                   num_devices=NCORES)

    bf16 = mybir.dt.bfloat16
    hT = nc.dram_tensor("hT", [D, S], bf16, kind="ExternalInput")
    wqT = nc.dram_tensor("wqT", [D, GD], bf16, kind="ExternalInput")
    wkT = nc.dram_tensor("wkT", [D, GD], bf16, kind="ExternalInput")
    wvT = nc.dram_tensor("wvT", [D, GD], bf16, kind="ExternalInput")
    woT = nc.dram_tensor("woT", [GD, D], bf16, kind="ExternalInput")
    out = nc.dram_tensor("out", [S, D], bf16, kind="ExternalOutput")

    with tile.TileContext(nc) as tc:
        for rep in range(reps):
            # reps>1 is a timing aid only: bodies serialize through SBUF
            # reuse, so t(reps=3)-t(reps=1) isolates two body executions.
            _emit_body(nc, tc, tile, mybir, hT, wqT, wkT, wvT, woT, out)

    nc.compile()
    return nc


def _emit_body(nc, tc, tile, mybir, hT, wqT, wkT, wvT, woT, out):
    f32 = mybir.dt.float32
    bf16 = mybir.dt.bfloat16
    Exp = mybir.ActivationFunctionType.Exp

    if True:
        with tc.tile_pool(name="persist", bufs=1) as persist:
            # per-head attention operands, filled by phase 1, all bf16:
            #   qt_all [e, h, tok]      (rhs of scores)
            #   kt_all [e, h, kb, tok]  (lhsT of scores)
            #   vt_all [tok, kb, h, hd] (lhsT of PV)
            qt_all = persist.tile([128, GH, S], bf16)
            kt_all = persist.tile([128, GH, NSB, 128], bf16)
            vt_all = persist.tile([128, NSB, GH, HD], bf16)
            wo_sb = persist.tile([128, GH, D], bf16)
            # f32r (E8M11) runs at 1 cycle/row at free>=256, vs 4 for true
            # f32; DVE writes into f32r tiles round on store, satisfying
            # the BIR verifier's rounded-producer rule.
            f32r = mybir.dt.float32r
            ones_f = persist.tile([128, 128], f32)
            ones_sb = persist.tile([128, 128], f32r)
            nc.vector.memset(ones_f, 1.0)
            nc.vector.tensor_copy(ones_sb, ones_f)

            # ---------------- phase 1: q/k/v projections ----------------
            with tc.tile_pool(name="wqk", bufs=1) as wqk, \
                 tc.tile_pool(name="hall", bufs=1) as hall, \
                 tc.tile_pool(name="ps1", bufs=8, space="PSUM") as ps1:
                wq_sb = wqk.tile([128, NEB, GD], bf16)
                wk_sb = wqk.tile([128, NEB, GD], bf16)
                wv_sb = wqk.tile([128, NEB, GD], bf16)
                h_all = hall.tile([128, NEB, S], bf16)
                wqT_r = wqT.rearrange("(n p) d -> p n d", p=128)
                wkT_r = wkT.rearrange("(n p) d -> p n d", p=128)
                wvT_r = wvT.rearrange("(n p) d -> p n d", p=128)
                woT_r = woT.rearrange("(n p) o -> p n o", p=128)
                hT_r = hT.rearrange("(n p) s -> p n s", p=128)

                # interleave so eb=0 pieces of the q/k path arrive first
                for eb in range(NEB):
                    nc.sync.dma_start(out=wq_sb[:, eb, :], in_=wqT_r[:, eb, :])
                    nc.sync.dma_start(out=wk_sb[:, eb, :], in_=wkT_r[:, eb, :])
                    nc.sync.dma_start(out=h_all[:, eb, :], in_=hT_r[:, eb, :])
                for eb in range(NEB):
                    nc.sync.dma_start(out=wv_sb[:, eb, :], in_=wvT_r[:, eb, :])
                for cb in range(GH):
                    nc.sync.dma_start(out=wo_sb[:, cb, :], in_=woT_r[:, cb, :])

                for st in range(NST):
                    tok = slice(st * ST, (st + 1) * ST)
                    if st == 0:
                        # eb-outer over 8 live accumulators: consume input
                        # chunks in arrival order so the PE tracks the DMA
                        # stream instead of stalling per accumulation.
                        pss = {}
                        for h in range(GH):
                            for t in range(2):
                                pss[(h, t)] = ps1.tile([128, ST], f32,
                                                       tag="ps1",
                                                       name=f"psqk{h}{t}")
                        for eb in range(NEB):
                            for h in range(GH):
                                for t, w_sb in ((0, wq_sb), (1, wk_sb)):
                                    nc.tensor.matmul(
                                        pss[(h, t)],
                                        w_sb[:, eb, h * HD:(h + 1) * HD],
                                        h_all[:, eb, tok],
                                        start=(eb == 0), stop=(eb == NEB - 1))
                        for h in range(GH):
                            nc.scalar.copy(qt_all[:, h, tok], pss[(h, 0)])
                            nc.scalar.copy(kt_all[:, h, st * 4:(st + 1) * 4, :],
                                           pss[(h, 1)])
                    else:
                        for h in range(GH):
                            for w_sb in (wq_sb, wk_sb):
                                ps = ps1.tile([128, ST], f32, tag="ps1")
                                for eb in range(NEB):
                                    nc.tensor.matmul(
                                        ps, w_sb[:, eb, h * HD:(h + 1) * HD],
                                        h_all[:, eb, tok],
                                        start=(eb == 0), stop=(eb == NEB - 1))
                                if w_sb is wq_sb:
                                    nc.scalar.copy(qt_all[:, h, tok], ps)
                                else:
                                    nc.scalar.copy(
                                        kt_all[:, h, st * 4:(st + 1) * 4, :],
                                        ps)
                    for j in range(ST // 128):
                        ps = ps1.tile([128, GD], f32, tag="ps1")
                        for eb in range(NEB):
                            nc.tensor.matmul(
                                ps,
                                h_all[:, eb, st * ST + j * 128:
                                      st * ST + (j + 1) * 128],
                                wv_sb[:, eb, :],
                                start=(eb == 0), stop=(eb == NEB - 1))
                        nc.scalar.copy(vt_all[:, st * 4 + j, :, :], ps)

            # ------- phase 2+3: attention fused with output projection -------
            # qt outer / heads inner: the output projection for query tile qt
            # runs as soon as all heads finished that tile, filling the PE
            # while the (ACT-bound) exp stream of the next tile runs.
            with tc.tile_pool(name="expp", bufs=2) as expp, \
                 tc.tile_pool(name="sm", bufs=2) as sm, \
                 tc.tile_pool(name="attn2", bufs=2) as attn2, \
                 tc.tile_pool(name="ev3", bufs=2) as ev3, \
                 tc.tile_pool(name="ps_s", bufs=2, space="PSUM") as ps_s, \
                 tc.tile_pool(name="acc", bufs=4, space="PSUM") as acc:

                def ph3_block(prev, qt_prev, j):
                    # output projection for query tile qt_prev, s-block j;
                    # evictions on DVE (ACT is pacing the exp stream) and
                    # the out DMA issued per 512-col chunk so nothing waits
                    # for the whole row block.
                    sb = qt_prev * (ST // 128) + j
                    rows = slice(sb * 128, (sb + 1) * 128)
                    ov = ev3.tile([128, D], bf16, tag="ov", name=f"ov{sb}")
                    for ot in range(NST):
                        ots = slice(ot * ST, (ot + 1) * ST)
                        po = acc.tile([128, ST], f32, tag="acc",
                                      name=f"po{sb}{ot}")
                        for cb in range(GH):
                            nc.tensor.matmul(
                                po,
                                prev[cb][:, j * 128:(j + 1) * 128],
                                wo_sb[:, cb, ots],
                                start=(cb == 0), stop=(cb == GH - 1))
                        nc.vector.tensor_copy(ov[:, ots], po)
                        nc.sync.dma_start(out=out[rows, ots], in_=ov[:, ots])

                pending = []

                def flush_den():
                    # denominator finish for the previous head, delayed so
                    # the DVE/Pool es-chains drain behind this head's first
                    # score pairs instead of stalling the PE.
                    es, esp, pa, at = pending.pop()
                    nc.vector.tensor_add(es, es.bitcast(f32), esp)
                    den = acc.tile([128, ST], f32, tag="acc", name="den")
                    nc.tensor.matmul(den, ones_sb, es, start=True, stop=True)
                    brc = sm.tile([128, ST], f32, tag="brc")
                    nc.vector.reciprocal(brc, den)
                    nc.vector.tensor_mul(at, pa, brc)

                prev_attn = None
                for qt in range(NST):
                    attn_t = []
                    for h in range(GH):
                        qs = qt_all[:, h, qt * ST:(qt + 1) * ST]
                        expt = expp.tile([128, NSB, ST], bf16, tag="expt")
                        pa = acc.tile([128, ST], f32, tag="acc", name="pa")
                        es = sm.tile([128, ST], f32r, tag="es")
                        esp = sm.tile([128, ST], f32, tag="esp")

                        # software-pipelined: scores for pair kb2+1 are
                        # enqueued before the exp-consumers of pair kb2 so
                        # the PE never waits on the ACT round-trip.
                        def scores_pair(kb2):
                            ps = ps_s.tile([128, 2, ST], f32, tag="ps")
                            for half in range(2):
                                nc.tensor.matmul(
                                    ps[:, half, :],
                                    kt_all[:, h, kb2 * 2 + half, :], qs,
                                    start=True, stop=True)
                            return ps

                        ps_prev = scores_pair(0)
                        for kb2 in range(NSB // 2):
                            kbo = kb2 * 2
                            ps_next = (scores_pair(kb2 + 1)
                                       if kb2 + 1 < NSB // 2 else None)
                            pair = expt[:, kbo:kbo + 2, :]
                            nc.scalar.activation(pair, ps_prev, Exp,
                                                 scale=SCALE)
                            if kb2 == 0 and pending:
                                flush_den()
                            # es-chain alternates pairs between DVE and
                            # Pool (gpsimd) so neither engine saturates
                            for half in range(2):
                                kb = kbo + half
                                if kb2 % 2 == 0:
                                    if kb == 0:
                                        nc.vector.tensor_copy(
                                            es, expt[:, kb, :])
                                    else:
                                        nc.vector.tensor_add(
                                            es, es.bitcast(f32),
                                            expt[:, kb, :])
                                else:
                                    if kb2 == 1 and half == 0:
                                        nc.gpsimd.tensor_copy(
                                            esp, expt[:, kb, :])
                                    else:
                                        nc.gpsimd.tensor_add(
                                            esp, esp, expt[:, kb, :])
                                nc.tensor.matmul(pa, vt_all[:, kb, h, :],
                                                 expt[:, kb, :],
                                                 start=(kb == 0),
                                                 stop=(kb == NSB - 1))
                            ps_prev = ps_next
                        at = attn2.tile([128, ST], bf16, tag=f"at{h}",
                                        name=f"at{h}{qt}")
                        pending.append((es, esp, pa, at))
                        attn_t.append(at)
                        if prev_attn is not None:
                            ph3_block(prev_attn, qt - 1, h)
                    prev_attn = attn_t
                flush_den()
                for j in range(ST // 128):
                    ph3_block(prev_attn, NST - 1, j)


def _get_runner(reps=1):
    if reps not in _RUNNER:
        _RUNNER[reps] = _build_nc(reps)
    return _RUNNER[reps]


def _bf16(x: np.ndarray) -> np.ndarray:
    import ml_dtypes
    return np.ascontiguousarray(x).astype(ml_dtypes.bfloat16)


def _prepare_in_maps(hidden_states, Wq, Wk, Wv, Wo):
    hidden = np.asarray(hidden_states, dtype=np.float32)
    hT = [_bf16(hidden[b].T) for b in range(B)]
    wq = np.asarray(Wq, dtype=np.float32)
    wk = np.asarray(Wk, dtype=np.float32)
    wv = np.asarray(Wv, dtype=np.float32)
    wo = np.asarray(Wo, dtype=np.float32)
    in_maps = []
    for core in range(NCORES):
        b, g = divmod(core, GROUPS)
        rows = slice(g * GD, (g + 1) * GD)
        in_maps.append({
            "hT": hT[b],
            "wqT": _bf16(wq[rows, :].T),
            "wkT": _bf16(wk[rows, :].T),
            "wvT": _bf16(wv[rows, :].T),
            "woT": _bf16(wo[:, rows].T),
        })
    return in_maps


def _run_device(in_maps, trace=False):
    from concourse.bass_utils import run_bass_kernel_spmd
    nc = _get_runner()
    try:
        return run_bass_kernel_spmd(nc, in_maps, core_ids=list(range(NCORES)),
                                    trace=trace)
    except Exception:
        # Transient device failures (rare) are recoverable by reopening the
        # backend with NEURON_RT_RESET_CORES=1. Retry once.
        try:
            import jax
            jax.clear_caches()
            try:
                jax.extend.backend.clear_backends()
            except Exception:
                jax._src.api.clear_backends()
        except Exception:
            pass
        return run_bass_kernel_spmd(nc, in_maps, core_ids=list(range(NCORES)),
                                    trace=trace)


def _numpy_reference(hidden_states, attention_mask, Wq, bq, Wk, bk, Wv, bv,
                     Wo, bo):
    """Exact fallback for inputs the fast path does not handle."""
    h = np.asarray(hidden_states, dtype=np.float32)
    mask = np.asarray(attention_mask)
    q = h @ np.asarray(Wq, np.float32).T + np.asarray(bq, np.float32)
    k = h @ np.asarray(Wk, np.float32).T + np.asarray(bk, np.float32)
    v = h @ np.asarray(Wv, np.float32).T + np.asarray(bv, np.float32)
    q = q.reshape(B, S, H, HD).transpose(0, 2, 1, 3)
    k = k.reshape(B, S, H, HD).transpose(0, 2, 1, 3)
    v = v.reshape(B, S, H, HD).transpose(0, 2, 1, 3)
    scores = (q @ k.transpose(0, 1, 3, 2)).astype(np.float32) * SCALE
    scores = np.where(mask == 0, np.float32(-1e9), scores)
    scores -= scores.max(axis=-1, keepdims=True)
    probs = np.exp(scores, dtype=np.float32)
    probs /= probs.sum(axis=-1, keepdims=True)
    attn = probs @ v
    attn = attn.transpose(0, 2, 1, 3).reshape(B, S, D)
    out = attn @ np.asarray(Wo, np.float32).T + np.asarray(bo, np.float32)
    return out.astype(np.float32)


def kernel(hidden_states, attention_mask, Wq, bq, Wk, bk, Wv, bv, Wo, bo):
    mask = np.asarray(attention_mask)
    bq_np = np.asarray(bq, dtype=np.float32)
    if (mask == 0).any() or np.any(bq_np):
        # general (never hit with the reference setup_inputs): bq shifts
        # scores per-key and a masked key changes the softmax support --
        # neither is representable in the fast path's fused layout.
        return _numpy_reference(hidden_states, attention_mask, Wq, bq, Wk,
                                bk, Wv, bv, Wo, bo)

    in_maps = _prepare_in_maps(hidden_states, Wq, Wk, Wv, Wo)
    res = _run_device(in_maps)

    # bk only adds a per-query constant to scores (softmax-invariant).
    # bv passes through the probs (rows sum to 1): out += bv @ Wo.T. bo adds.
    extra = (np.asarray(bv, np.float64) @ np.asarray(Wo, np.float64).T
             + np.asarray(bo, np.float64))
    out = np.empty((B, S, D), dtype=np.float32)
    for b in range(B):
        acc = np.zeros((S, D), dtype=np.float64)
        for g in range(GROUPS):
            acc += np.asarray(res.results[b * GROUPS + g]["out"],
                              dtype=np.float64)
        out[b] = (acc + extra).astype(np.float32)
    return out
